# revision 5
# baseline (speedup 1.0000x reference)
"""Trainium2 Bass kernel for GQA attention block (B=2,T=2048,D=2048,H=16,KV=4,HD=128).

Sharding: 8 cores = 2 batches x 4 kv-groups. Core c handles batch b=c//4 and
kv-head g=c%4 (q-heads 4g..4g+3). wq/wk/wv column-sharded, wo row-sharded;
partial outputs are summed on the host (4 partials per batch).

Per-core dataflow (all matmuls in fp32r, 1 cycle/row):
  phase 1: qT/kT/vT projections from host-pretransposed x, fused
           RMSNorm (ACT Square+accum, Sqrt, DVE recip) + RoPE
           (scalar_tensor_tensor with host-folded cos/sin tables incl.
           qn/kn scale and rotation sign), PE-transpose into [hd, t] layout.
  phase 2: per (head, q-chunk): scores^T = kT.T@qT -> ACT tanh (cap) ->
           ACT exp -> optional mask multiply -> PE attn@v accumulate +
           ones-matmul row sums; normalize by DVE reciprocal + DMA
           partition-broadcast.
  phase 3: out^T heads @ wo row-block -> partial [T, D] output.

The causal/arbitrary mask is handled by host-side block classification:
all-false blocks are skipped, all-true blocks skip the mask multiply,
mixed blocks multiply by a packed 0/1 f32 tile.
"""

import math

import numpy as np

import concourse.bass as bass
import concourse.mybir as mybir
import concourse.tile as tile
from concourse import bacc
from concourse.bass_utils import run_bass_kernel_spmd

F32 = mybir.dt.float32
F32R = mybir.dt.float32r
AF = mybir.ActivationFunctionType
OP = mybir.AluOpType

B, T, D = 2, 2048, 2048
H, KV, HD = 16, 4, 128
GROUPS = H // KV
CAP = 50.0
EPS = 1e-6
NCORES = 8

TCH = 128  # t-chunk (phase-1 M, outproj M)
QC = 512   # q-chunk (phase-2 N); must be >=256 for fp32r full rate


def _build_nc(t_len, plan, n_mixed, reps=1):
    """Build the per-core Bass program.

    plan: list over q-chunks of list of (kt, mixed_idx) with mixed_idx=-1
    for all-true blocks. n_mixed: number of packed mask tiles (>=1 dummy).
    """
    n_tc = t_len // TCH
    n_qc = t_len // QC
    n_dt = D // 128
    d_heads = GROUPS  # q heads per core

    nc = bacc.Bacc("TRN2", target_bir_lowering=False, debug=False,
                   num_devices=NCORES)

    xh = nc.dram_tensor("xh", [n_tc, 128, n_dt * TCH], F32, kind="ExternalInput")
    wq_d = nc.dram_tensor("wq", [D, d_heads * HD], F32, kind="ExternalInput")
    wkv_d = nc.dram_tensor("wkv", [D, 2 * HD], F32, kind="ExternalInput")
    wo_d = nc.dram_tensor("wo", [d_heads * HD, D], F32, kind="ExternalInput")
    cosq_d = nc.dram_tensor("cosq", [t_len, HD], F32, kind="ExternalInput")
    sinq_d = nc.dram_tensor("sinq", [t_len, HD], F32, kind="ExternalInput")
    cosk_d = nc.dram_tensor("cosk", [t_len, HD], F32, kind="ExternalInput")
    sink_d = nc.dram_tensor("sink", [t_len, HD], F32, kind="ExternalInput")
    ident_d = nc.dram_tensor("ident", [128, 128], F32, kind="ExternalInput")
    maskp_d = nc.dram_tensor("maskp", [max(n_mixed, 1), 128, QC], F32,
                             kind="ExternalInput")
    out_d = nc.dram_tensor("out_p", [t_len, D], F32, kind="ExternalOutput")
    rbounce_d = nc.dram_tensor("rbounce", [(t_len // QC) * GROUPS, QC], F32)

    inv_cap_scale = 1.0 / (CAP * math.sqrt(HD))

    with tile.TileContext(nc) as tc:
        for _rep in range(reps):
            _emit_body(nc, tc, t_len, plan, xh, wq_d, wkv_d, wo_d,
                       cosq_d, sinq_d, cosk_d, sink_d, ident_d, maskp_d,
                       out_d, rbounce_d, inv_cap_scale)
    nc.compile()
    return nc


def _emit_body(nc, tc, t_len, plan, xh, wq_d, wkv_d, wo_d, cosq_d, sinq_d,
               cosk_d, sink_d, ident_d, maskp_d, out_d, rbounce_d,
               inv_cap_scale):
    n_tc = t_len // TCH
    n_qc = t_len // QC
    n_dt = D // 128
    d_heads = GROUPS

    import contextlib
    with contextlib.ExitStack() as ctx:
        persist = ctx.enter_context(tc.tile_pool(name="persist", bufs=1))

        # persistent stores
        qT_all = persist.tile([128, d_heads, t_len], F32R, tag="qT_all")
        kT_all = persist.tile([128, t_len], F32R, tag="kT_all")
        v_all = persist.tile([128, n_tc, HD], F32R, tag="v_all")
        ident = persist.tile([128, 128], F32, tag="ident")
        nc.sync.dma_start(out=ident, in_=ident_d.ap())
        ones_f = persist.tile([128, 1], F32, tag="ones_f")
        nc.vector.memset(ones_f, 1.0)
        ones_t = persist.tile([128, 1], F32R, tag="ones")
        nc.scalar.copy(out=ones_t, in_=ones_f)
        eps_t = persist.tile([128, 1], F32, tag="eps")
        nc.vector.memset(eps_t, EPS)

        # ---------------- phase 1: projections + norm + rope ----------
        with tc.tile_pool(name="w1", bufs=1) as w1, \
             tc.tile_pool(name="xstage", bufs=2) as xstage, \
             tc.tile_pool(name="cs", bufs=2) as cs, \
             tc.tile_pool(name="p1sm", bufs=4) as p1sm, \
             tc.tile_pool(name="p1work", bufs=3) as p1work, \
             tc.tile_pool(name="psq", bufs=2, space="PSUM") as psq, \
             tc.tile_pool(name="pskv", bufs=2, space="PSUM") as pskv, \
             tc.tile_pool(name="pstr", bufs=2, space="PSUM") as pstr:

            wq_r = w1.tile([128, n_dt, d_heads * HD], F32R, tag="wq")
            wkv_r = w1.tile([128, n_dt, 2 * HD], F32R, tag="wkv")
            with tc.tile_pool(name="wstage", bufs=3) as wstage:
                for dt in range(n_dt):
                    st = wstage.tile([128, d_heads * HD], F32, tag="wst")
                    nc.sync.dma_start(
                        out=st, in_=wq_d.ap()[dt * 128:(dt + 1) * 128, :])
                    nc.scalar.copy(out=wq_r[:, dt, :], in_=st)
                    st2 = wstage.tile([128, 2 * HD], F32, tag="wst2")
                    nc.sync.dma_start(
                        out=st2, in_=wkv_d.ap()[dt * 128:(dt + 1) * 128, :])
                    nc.scalar.copy(out=wkv_r[:, dt, :], in_=st2)

            def norm_rope_transpose(psrc, cos_t, sin_t, dst_slice):
                # psrc: PSUM [128, 128] (t x hd) raw projection for one head
                scratch = p1work.tile([128, HD], F32, tag="sqscr")
                ssq = p1work.tile([128, 1], F32, tag="ssq")
                nc.scalar.activation(scratch, psrc, AF.Square, accum_out=ssq)
                std = p1work.tile([128, 1], F32, tag="std")
                nc.scalar.activation(std, ssq, AF.Sqrt, scale=1.0 / HD,
                                     bias=eps_t)
                rinv = p1work.tile([128, 1], F32, tag="rinv")
                nc.vector.reciprocal(rinv, std)
                r1 = p1work.tile([128, HD], F32, tag="r1")
                nc.vector.scalar_tensor_tensor(r1, psrc, rinv, cos_t,
                                               op0=OP.mult, op1=OP.mult)
                qr = p1work.tile([128, HD], F32, tag="qr")
                p2 = psrc.rearrange("p (f two) -> p f two", two=2)
                s2 = sin_t.rearrange("p (f two) -> p f two", two=2)
                q2 = qr.rearrange("p (f two) -> p f two", two=2)
                nc.vector.scalar_tensor_tensor(q2[:, :, 0], p2[:, :, 1], rinv,
                                               s2[:, :, 0],
                                               op0=OP.mult, op1=OP.mult)
                nc.vector.scalar_tensor_tensor(q2[:, :, 1], p2[:, :, 0], rinv,
                                               s2[:, :, 1],
                                               op0=OP.mult, op1=OP.mult)
                nc.vector.tensor_add(qr, qr, r1)
                ptr = pstr.tile([128, 128], F32, tag="ptr")
                nc.tensor.transpose(ptr, qr, ident)
                nc.vector.tensor_copy(dst_slice, ptr)

            for tci in range(n_tc):
                xt = xstage.tile([128, n_dt * TCH], F32, tag="xt")
                nc.sync.dma_start(out=xt, in_=xh.ap()[tci])
                xrt = xstage.tile([128, n_dt * TCH], F32R, tag="xr")
                nc.scalar.copy(out=xrt, in_=xt)
                xr = xrt.rearrange("p (dt t) -> p dt t", dt=n_dt)

                pq = psq.tile([128, d_heads * HD], F32, tag="pq")
                pkv = pskv.tile([128, 2 * HD], F32, tag="pkv")
                for dt in range(n_dt):
                    nc.tensor.matmul(pq, xr[:, dt, :], wq_r[:, dt, :],
                                     start=(dt == 0), stop=(dt == n_dt - 1))
                for dt in range(n_dt):
                    nc.tensor.matmul(pkv, xr[:, dt, :], wkv_r[:, dt, :],
                                     start=(dt == 0), stop=(dt == n_dt - 1))

                t0 = tci * TCH
                cq = cs.tile([128, HD], F32, tag="cq")
                nc.sync.dma_start(out=cq, in_=cosq_d.ap()[t0:t0 + TCH, :])
                sq_ = cs.tile([128, HD], F32, tag="sq")
                nc.sync.dma_start(out=sq_, in_=sinq_d.ap()[t0:t0 + TCH, :])
                ck = cs.tile([128, HD], F32, tag="ck")
                nc.sync.dma_start(out=ck, in_=cosk_d.ap()[t0:t0 + TCH, :])
                sk = cs.tile([128, HD], F32, tag="sk")
                nc.sync.dma_start(out=sk, in_=sink_d.ap()[t0:t0 + TCH, :])

                for h in range(d_heads):
                    norm_rope_transpose(pq[:, h * HD:(h + 1) * HD], cq, sq_,
                                        qT_all[:, h, t0:t0 + TCH])
                norm_rope_transpose(pkv[:, 0:HD], ck, sk,
                                    kT_all[:, t0:t0 + TCH])
                nc.scalar.copy(out=v_all[:, tci, :], in_=pkv[:, HD:2 * HD])

        # ---------------- phase 2+3: attention + output projection ----
        with tc.tile_pool(name="w2", bufs=1) as w2, \
             tc.tile_pool(name="p2work", bufs=3) as p2work, \
             tc.tile_pool(name="pTpool", bufs=3) as pTpool, \
             tc.tile_pool(name="mstage", bufs=2) as mstage, \
             tc.tile_pool(name="ohpool", bufs=2) as ohpool, \
             tc.tile_pool(name="norm2", bufs=2) as norm2, \
             tc.tile_pool(name="obuf", bufs=3) as obuf, \
             tc.tile_pool(name="pss", bufs=2, space="PSUM") as pss, \
             tc.tile_pool(name="psav", bufs=2, space="PSUM") as psav, \
             tc.tile_pool(name="psrs", bufs=2, space="PSUM") as psrs, \
             tc.tile_pool(name="psop", bufs=2, space="PSUM") as psop:

            wo_r = w2.tile([128, d_heads, D], F32R, tag="wo")
            with tc.tile_pool(name="wstage2", bufs=2) as wstage2:
                for h in range(d_heads):
                    st = wstage2.tile([128, D], F32, tag="wost")
                    nc.sync.dma_start(
                        out=st, in_=wo_d.ap()[h * 128:(h + 1) * 128, :])
                    nc.scalar.copy(out=wo_r[:, h, :], in_=st)

            for qci in range(n_qc):
                q0 = qci * QC
                oh_all = ohpool.tile([128, d_heads, QC], F32R, tag="oh")
                for h in range(d_heads):
                    kts = plan[qci]
                    pav = psav.tile([128, QC], F32, tag="pav")
                    prs = psrs.tile([1, QC], F32, tag="prs")
                    for idx, (kt, mix) in enumerate(kts):
                        ps = pss.tile([128, QC], F32, tag="ps")
                        nc.tensor.matmul(
                            ps, kT_all[:, kt * 128:(kt + 1) * 128],
                            qT_all[:, h, q0:q0 + QC],
                            start=True, stop=True)
                        th = p2work.tile([128, QC], F32, tag="th")
                        nc.scalar.activation(th, ps, AF.Tanh,
                                             scale=inv_cap_scale)
                        first, last = (idx == 0), (idx == len(kts) - 1)
                        if mix < 0:
                            pt = pTpool.tile([128, QC], F32R, tag="pt")
                            nc.scalar.activation(pt, th, AF.Exp, scale=CAP)
                        else:
                            ptf = p2work.tile([128, QC], F32, tag="ptf")
                            nc.scalar.activation(ptf, th, AF.Exp, scale=CAP)
                            mt = mstage.tile([128, QC], F32, tag="mt")
                            nc.sync.dma_start(out=mt, in_=maskp_d.ap()[mix])
                            pt = pTpool.tile([128, QC], F32R, tag="pt")
                            nc.vector.tensor_tensor(pt, ptf, mt, op=OP.mult)
                        nc.tensor.matmul(pav, v_all[:, kt, :], pt,
                                         start=first, stop=last)
                        nc.tensor.matmul(prs, ones_t, pt,
                                         start=first, stop=last)
                    # normalize: oh = pav / rowsum (broadcast along hd)
                    rin = norm2.tile([1, QC], F32, tag="rin")
                    nc.vector.reciprocal(rin, prs)
                    slot = qci * GROUPS + h
                    nc.sync.dma_start(out=rbounce_d.ap()[slot:slot + 1, :],
                                      in_=rin)
                    rb = norm2.tile([128, QC], F32, tag="rb")
                    bsrc = rbounce_d.ap()[slot, :]
                    nc.gpsimd.dma_start(
                        out=rb,
                        in_=bass.AP(tensor=bsrc.tensor, offset=bsrc.offset,
                                    ap=[[0, 128]] + list(bsrc.ap)))
                    nc.vector.tensor_tensor(oh_all[:, h, :], pav, rb,
                                            op=OP.mult)

                # outproj for this q-chunk
                for tsub in range(QC // TCH):
                    for dc in range(D // 512):
                        pop = psop.tile([128, 512], F32, tag="pop")
                        for h in range(d_heads):
                            nc.tensor.matmul(
                                pop,
                                oh_all[:, h, tsub * TCH:(tsub + 1) * TCH],
                                wo_r[:, h, dc * 512:(dc + 1) * 512],
                                start=(h == 0), stop=(h == d_heads - 1))
                        ot = obuf.tile([128, 512], F32, tag="ot")
                        nc.vector.tensor_copy(ot, pop)
                        r0 = q0 + tsub * TCH
                        nc.sync.dma_start(
                            out=out_d.ap()[r0:r0 + TCH,
                                           dc * 512:(dc + 1) * 512],
                            in_=ot)


def _host_prep(x, wq, wk, wv, wo, qn_scale, kn_scale, cos, sin, mask, t_len):
    """Build per-core input maps + the block plan from the mask."""
    n_tc = t_len // TCH
    n_dt = D // 128
    n_qc = t_len // QC

    # fused rope tables (scale fold + rotation sign fold)
    def fuse(scale):
        s = np.asarray(scale, dtype=np.float32)
        c = np.asarray(cos, dtype=np.float32) * s[None, :]
        sw = np.empty_like(s)
        sw[0::2] = s[1::2]
        sw[1::2] = s[0::2]
        sn = np.asarray(sin, dtype=np.float32) * sw[None, :]
        sn[:, 0::2] *= -1.0
        return np.ascontiguousarray(c), np.ascontiguousarray(sn)

    cosq, sinq = fuse(qn_scale)
    cosk, sink = fuse(kn_scale)

    # mask block classification (mask: [1,1,T,T] bool, rows=q, cols=k)
    m2 = np.asarray(mask).reshape(t_len, t_len)
    plan = []
    mix_tiles = []
    for qci in range(n_qc):
        row = []
        qs = slice(qci * QC, (qci + 1) * QC)
        for kt in range(n_tc):
            blk = m2[qs, kt * 128:(kt + 1) * 128]
            if not blk.any():
                continue
            if blk.all():
                row.append((kt, -1))
            else:
                mix_tiles.append(np.ascontiguousarray(
                    blk.T.astype(np.float32)))
                row.append((kt, len(mix_tiles) - 1))
        plan.append(row)
    maskp = (np.stack(mix_tiles) if mix_tiles
             else np.zeros((1, 128, QC), np.float32))

    ident = np.eye(128, dtype=np.float32)
    in_maps = []
    for c in range(NCORES):
        b, g = divmod(c, GROUPS)
        xt = np.asarray(x[b], dtype=np.float32).T  # [D, T]
        # xh[tc, p, dt*TCH + t] = xT[dt*128+p, tc*TCH+t]
        xhc = np.ascontiguousarray(
            xt.reshape(n_dt, 128, n_tc, TCH).transpose(2, 1, 0, 3)
            .reshape(n_tc, 128, n_dt * TCH))
        h0 = g * GROUPS
        in_maps.append({
            "xh": xhc,
            "wq": np.ascontiguousarray(
                np.asarray(wq, np.float32)[:, h0 * HD:(h0 + GROUPS) * HD]),
            "wkv": np.ascontiguousarray(np.concatenate(
                [np.asarray(wk, np.float32)[:, g * HD:(g + 1) * HD],
                 np.asarray(wv, np.float32)[:, g * HD:(g + 1) * HD]], axis=1)),
            "wo": np.ascontiguousarray(
                np.asarray(wo, np.float32)[h0 * HD:(h0 + GROUPS) * HD, :]),
            "cosq": cosq, "sinq": sinq, "cosk": cosk, "sink": sink,
            "ident": ident, "maskp": maskp,
        })
    return in_maps, plan, len(mix_tiles)


_NC_CACHE = {}


def _plan_key(plan, n_mixed, t_len, reps):
    return (t_len, n_mixed, reps,
            tuple(tuple(r) for r in plan))


def kernel(x, wq, wk, wv, wo, qn_scale, kn_scale, cos, sin, mask):
    t_len = x.shape[1]
    in_maps, plan, n_mixed = _host_prep(
        x, wq, wk, wv, wo, qn_scale, kn_scale, cos, sin, mask, t_len)
    key = _plan_key(plan, n_mixed, t_len, 1)
    nc = _NC_CACHE.get(key)
    if nc is None:
        nc = _build_nc(t_len, plan, n_mixed)
        _NC_CACHE[key] = nc
    res = run_bass_kernel_spmd(nc, in_maps, list(range(NCORES)))
    out = np.zeros((B, t_len, D), dtype=np.float32)
    for c in range(NCORES):
        out[c // GROUPS] += res.results[c]["out_p"]
    return out


# revision 6
# speedup vs baseline: 1.0283x; 1.0283x over previous
"""Trainium2 Bass kernel for GQA attention block (B=2,T=2048,D=2048,H=16,KV=4,HD=128).

Sharding: 8 cores = 2 batches x 4 kv-groups. Core c handles batch b=c//4 and
kv-head g=c%4 (q-heads 4g..4g+3). wq/wk/wv column-sharded, wo row-sharded;
partial outputs are summed on the host (4 partials per batch).

Per-core dataflow (all matmuls in fp32r, 1 cycle/row; fp32r operands that
come from DRAM are pre-truncated on the host, which is idempotent under the
hardware rounding):
  phase 1: qT/kT/vT projections from host-pretransposed x, fused
           RMSNorm (ACT Square+accum, Sqrt, DVE recip) + RoPE
           (scalar_tensor_tensor with host-folded cos/sin tables incl.
           qn/kn scale and rotation sign), PE-transpose into [hd, t] layout.
  phase 2: per (head, q-chunk): scores^T = kT.T@qT (pairs of k-tiles into a
           2-bank PSUM tile) -> one ACT tanh + one ACT exp per pair ->
           optional mask multiply -> PE attn@v accumulate + ones-matmul row
           sums; normalize via DVE reciprocal + DRAM-bounce broadcast.
  phase 3: out^T heads @ wo row-block -> partial [T, D] output.

The causal/arbitrary mask is handled by host-side block classification:
all-false blocks are skipped, all-true blocks skip the mask multiply,
mixed blocks multiply by a packed 0/1 f32 tile.
"""

import math

import numpy as np

import concourse.bass as bass
import concourse.mybir as mybir
import concourse.tile as tile
from concourse import bacc
from concourse.bass_utils import run_bass_kernel_spmd

F32 = mybir.dt.float32
F32R = mybir.dt.float32r
AF = mybir.ActivationFunctionType
OP = mybir.AluOpType

B, T, D = 2, 2048, 2048
H, KV, HD = 16, 4, 128
GROUPS = H // KV
CAP = 50.0
EPS = 1e-6
NCORES = 8

TCH = 128  # t-chunk (phase-1 M, outproj M)
QC = 512   # q-chunk (phase-2 N); must be >=256 for fp32r full rate
KGRP = 2   # k-tiles per PSUM/ACT batch in phase 2


def _trunc_f32r(a):
    """Host-side fp32r pre-rounding (truncate low 11 mantissa bits)."""
    return (np.ascontiguousarray(a, dtype=np.float32)
            .view(np.uint32) & np.uint32(0xFFFFF800)).view(np.float32)


def _build_nc(t_len, plan, n_mixed, reps=1):
    """plan: per q-chunk, a list of groups; each group is a list of
    (kt, mixed_idx) pairs (mixed_idx=-1 for all-true blocks)."""
    n_tc = t_len // TCH
    n_dt = D // 128

    nc = bacc.Bacc("TRN2", target_bir_lowering=False, debug=False,
                   num_devices=NCORES)

    xh = nc.dram_tensor("xh", [n_tc, 128, n_dt * TCH], F32R,
                        kind="ExternalInput")
    wq_d = nc.dram_tensor("wq", [D, GROUPS * HD], F32R, kind="ExternalInput")
    wkv_d = nc.dram_tensor("wkv", [D, 2 * HD], F32R, kind="ExternalInput")
    wo_d = nc.dram_tensor("wo", [GROUPS * HD, D], F32R, kind="ExternalInput")
    cosq_d = nc.dram_tensor("cosq", [t_len, HD], F32, kind="ExternalInput")
    sinq_d = nc.dram_tensor("sinq", [t_len, HD], F32, kind="ExternalInput")
    cosk_d = nc.dram_tensor("cosk", [t_len, HD], F32, kind="ExternalInput")
    sink_d = nc.dram_tensor("sink", [t_len, HD], F32, kind="ExternalInput")
    ident_d = nc.dram_tensor("ident", [128, 128], F32, kind="ExternalInput")
    maskp_d = nc.dram_tensor("maskp", [max(n_mixed, 1), 128, QC], F32,
                             kind="ExternalInput")
    out_d = nc.dram_tensor("out_p", [t_len, D], F32, kind="ExternalOutput")
    rbounce_d = nc.dram_tensor("rbounce", [(t_len // QC) * GROUPS, QC], F32)

    inv_cap_scale = 1.0 / (CAP * math.sqrt(HD))

    with tile.TileContext(nc) as tc:
        for _rep in range(reps):
            _emit_body(nc, tc, t_len, plan, xh, wq_d, wkv_d, wo_d,
                       cosq_d, sinq_d, cosk_d, sink_d, ident_d, maskp_d,
                       out_d, rbounce_d, inv_cap_scale)
    nc.compile()
    return nc


def _emit_body(nc, tc, t_len, plan, xh, wq_d, wkv_d, wo_d, cosq_d, sinq_d,
               cosk_d, sink_d, ident_d, maskp_d, out_d, rbounce_d,
               inv_cap_scale):
    n_tc = t_len // TCH
    n_qc = t_len // QC
    n_dt = D // 128

    import contextlib
    with contextlib.ExitStack() as ctx:
        persist = ctx.enter_context(tc.tile_pool(name="persist", bufs=1))

        qT_all = persist.tile([128, GROUPS, t_len], F32R, tag="qT_all")
        kT_all = persist.tile([128, t_len], F32R, tag="kT_all")
        v_all = persist.tile([128, n_tc, HD], F32R, tag="v_all")
        ident = persist.tile([128, 128], F32, tag="ident")
        nc.sync.dma_start(out=ident, in_=ident_d.ap())
        ones_f = persist.tile([128, 1], F32, tag="ones_f")
        nc.vector.memset(ones_f, 1.0)
        ones_t = persist.tile([128, 1], F32R, tag="ones")
        nc.scalar.copy(out=ones_t, in_=ones_f)
        eps_t = persist.tile([128, 1], F32, tag="eps")
        nc.vector.memset(eps_t, EPS)

        # ---------------- phase 1: projections + norm + rope ----------
        with tc.tile_pool(name="w1", bufs=1) as w1, \
             tc.tile_pool(name="xstage", bufs=2) as xstage, \
             tc.tile_pool(name="cs", bufs=2) as cs, \
             tc.tile_pool(name="p1work", bufs=3) as p1work, \
             tc.tile_pool(name="psq", bufs=2, space="PSUM") as psq, \
             tc.tile_pool(name="pskv", bufs=2, space="PSUM") as pskv, \
             tc.tile_pool(name="pstr", bufs=2, space="PSUM") as pstr:

            wq_r = w1.tile([128, n_dt, GROUPS * HD], F32R, tag="wq")
            nc.sync.dma_start(
                out=wq_r,
                in_=wq_d.ap().rearrange("(dt p) n -> p dt n", p=128))
            wkv_r = w1.tile([128, n_dt, 2 * HD], F32R, tag="wkv")
            nc.sync.dma_start(
                out=wkv_r,
                in_=wkv_d.ap().rearrange("(dt p) n -> p dt n", p=128))

            def norm_rope_transpose(psrc, cos_t, sin_t, dst_slice):
                # psrc: PSUM [128, 128] (t x hd) raw projection for one head
                scratch = p1work.tile([128, HD], F32, tag="sqscr")
                ssq = p1work.tile([128, 1], F32, tag="ssq")
                nc.scalar.activation(scratch, psrc, AF.Square, accum_out=ssq)
                std = p1work.tile([128, 1], F32, tag="std")
                nc.scalar.activation(std, ssq, AF.Sqrt, scale=1.0 / HD,
                                     bias=eps_t)
                rinv = p1work.tile([128, 1], F32, tag="rinv")
                nc.vector.reciprocal(rinv, std)
                r1 = p1work.tile([128, HD], F32, tag="r1")
                nc.vector.scalar_tensor_tensor(r1, psrc, rinv, cos_t,
                                               op0=OP.mult, op1=OP.mult)
                qr = p1work.tile([128, HD], F32, tag="qr")
                p2 = psrc.rearrange("p (f two) -> p f two", two=2)
                s2 = sin_t.rearrange("p (f two) -> p f two", two=2)
                q2 = qr.rearrange("p (f two) -> p f two", two=2)
                nc.vector.scalar_tensor_tensor(q2[:, :, 0], p2[:, :, 1], rinv,
                                               s2[:, :, 0],
                                               op0=OP.mult, op1=OP.mult)
                nc.vector.scalar_tensor_tensor(q2[:, :, 1], p2[:, :, 0], rinv,
                                               s2[:, :, 1],
                                               op0=OP.mult, op1=OP.mult)
                nc.vector.tensor_add(qr, qr, r1)
                ptr = pstr.tile([128, 128], F32, tag="ptr")
                nc.tensor.transpose(ptr, qr, ident)
                nc.vector.tensor_copy(dst_slice, ptr)

            for tci in range(n_tc):
                xt = xstage.tile([128, n_dt * TCH], F32R, tag="xt")
                nc.sync.dma_start(out=xt, in_=xh.ap()[tci])
                xr = xt.rearrange("p (dt t) -> p dt t", dt=n_dt)

                pq = psq.tile([128, GROUPS * HD], F32, tag="pq")
                pkv = pskv.tile([128, 2 * HD], F32, tag="pkv")
                for dt in range(n_dt):
                    nc.tensor.matmul(pq, xr[:, dt, :], wq_r[:, dt, :],
                                     start=(dt == 0), stop=(dt == n_dt - 1))
                for dt in range(n_dt):
                    nc.tensor.matmul(pkv, xr[:, dt, :], wkv_r[:, dt, :],
                                     start=(dt == 0), stop=(dt == n_dt - 1))

                t0 = tci * TCH
                cq = cs.tile([128, HD], F32, tag="cq")
                nc.sync.dma_start(out=cq, in_=cosq_d.ap()[t0:t0 + TCH, :])
                sq_ = cs.tile([128, HD], F32, tag="sq")
                nc.sync.dma_start(out=sq_, in_=sinq_d.ap()[t0:t0 + TCH, :])
                ck = cs.tile([128, HD], F32, tag="ck")
                nc.sync.dma_start(out=ck, in_=cosk_d.ap()[t0:t0 + TCH, :])
                sk = cs.tile([128, HD], F32, tag="sk")
                nc.sync.dma_start(out=sk, in_=sink_d.ap()[t0:t0 + TCH, :])

                for h in range(GROUPS):
                    norm_rope_transpose(pq[:, h * HD:(h + 1) * HD], cq, sq_,
                                        qT_all[:, h, t0:t0 + TCH])
                norm_rope_transpose(pkv[:, 0:HD], ck, sk,
                                    kT_all[:, t0:t0 + TCH])
                nc.scalar.copy(out=v_all[:, tci, :], in_=pkv[:, HD:2 * HD])

        # ---------------- phase 2+3: attention + output projection ----
        with tc.tile_pool(name="w2", bufs=1) as w2, \
             tc.tile_pool(name="p2work", bufs=3) as p2work, \
             tc.tile_pool(name="pTpool", bufs=3) as pTpool, \
             tc.tile_pool(name="mstage", bufs=2) as mstage, \
             tc.tile_pool(name="ohpool", bufs=2) as ohpool, \
             tc.tile_pool(name="norm2", bufs=2) as norm2, \
             tc.tile_pool(name="obuf", bufs=3) as obuf, \
             tc.tile_pool(name="pss", bufs=2, space="PSUM") as pss, \
             tc.tile_pool(name="psav", bufs=1, space="PSUM") as psav, \
             tc.tile_pool(name="psrs", bufs=1, space="PSUM") as psrs, \
             tc.tile_pool(name="psop", bufs=2, space="PSUM") as psop:

            wo_r = w2.tile([128, GROUPS, D], F32R, tag="wo")
            nc.sync.dma_start(
                out=wo_r,
                in_=wo_d.ap().rearrange("(h p) n -> p h n", p=128))

            for qci in range(n_qc):
                q0 = qci * QC
                oh_all = ohpool.tile([128, GROUPS, QC], F32R, tag="oh")
                for h in range(GROUPS):
                    groups_ = plan[qci]
                    n_kt = sum(len(g) for g in groups_)
                    pav = psav.tile([128, QC], F32, tag="pav")
                    prs = psrs.tile([1, QC], F32, tag="prs")
                    gidx = 0
                    for grp in groups_:
                        gw = len(grp)
                        ps = pss.tile([128, KGRP * QC], F32, tag="ps")
                        for j, (kt, mix) in enumerate(grp):
                            nc.tensor.matmul(
                                ps[:, j * QC:(j + 1) * QC],
                                kT_all[:, kt * 128:(kt + 1) * 128],
                                qT_all[:, h, q0:q0 + QC],
                                start=True, stop=True)
                        pview = ps[:, 0:gw * QC]
                        th = p2work.tile([128, KGRP * QC], F32, tag="th")
                        thv = th[:, 0:gw * QC]
                        nc.scalar.activation(thv, pview, AF.Tanh,
                                             scale=inv_cap_scale)
                        any_mix = any(mix >= 0 for (_, mix) in grp)
                        pt = pTpool.tile([128, KGRP * QC], F32R, tag="pt")
                        if not any_mix:
                            nc.scalar.activation(pt[:, 0:gw * QC], thv,
                                                 AF.Exp, scale=CAP)
                        else:
                            ptf = p2work.tile([128, KGRP * QC], F32,
                                              tag="ptf")
                            nc.scalar.activation(ptf[:, 0:gw * QC], thv,
                                                 AF.Exp, scale=CAP)
                            for j, (kt, mix) in enumerate(grp):
                                sl = slice(j * QC, (j + 1) * QC)
                                if mix >= 0:
                                    mt = mstage.tile([128, QC], F32,
                                                     tag="mt")
                                    nc.sync.dma_start(
                                        out=mt, in_=maskp_d.ap()[mix])
                                    nc.vector.tensor_tensor(
                                        pt[:, sl], ptf[:, sl], mt,
                                        op=OP.mult)
                                else:
                                    nc.vector.tensor_copy(
                                        pt[:, sl], ptf[:, sl])
                        for j, (kt, mix) in enumerate(grp):
                            sl = slice(j * QC, (j + 1) * QC)
                            first = gidx == 0
                            last = gidx == n_kt - 1
                            nc.tensor.matmul(pav, v_all[:, kt, :],
                                             pt[:, sl],
                                             start=first, stop=last)
                            nc.tensor.matmul(prs, ones_t, pt[:, sl],
                                             start=first, stop=last)
                            gidx += 1
                    # normalize: oh = pav / rowsum (broadcast along hd)
                    rin = norm2.tile([1, QC], F32, tag="rin")
                    nc.vector.reciprocal(rin, prs)
                    slot = qci * GROUPS + h
                    nc.sync.dma_start(out=rbounce_d.ap()[slot:slot + 1, :],
                                      in_=rin)
                    rb = norm2.tile([128, QC], F32, tag="rb")
                    bsrc = rbounce_d.ap()[slot, :]
                    nc.gpsimd.dma_start(
                        out=rb,
                        in_=bass.AP(tensor=bsrc.tensor, offset=bsrc.offset,
                                    ap=[[0, 128]] + list(bsrc.ap)))
                    nc.vector.tensor_tensor(oh_all[:, h, :], pav, rb,
                                            op=OP.mult)

                # outproj for this q-chunk
                for tsub in range(QC // TCH):
                    for dc in range(D // 512):
                        pop = psop.tile([128, 512], F32, tag="pop")
                        for h in range(GROUPS):
                            nc.tensor.matmul(
                                pop,
                                oh_all[:, h, tsub * TCH:(tsub + 1) * TCH],
                                wo_r[:, h, dc * 512:(dc + 1) * 512],
                                start=(h == 0), stop=(h == GROUPS - 1))
                        ot = obuf.tile([128, 512], F32, tag="ot")
                        nc.vector.tensor_copy(ot, pop)
                        r0 = q0 + tsub * TCH
                        nc.sync.dma_start(
                            out=out_d.ap()[r0:r0 + TCH,
                                           dc * 512:(dc + 1) * 512],
                            in_=ot)


def _host_prep(x, wq, wk, wv, wo, qn_scale, kn_scale, cos, sin, mask, t_len):
    """Build per-core input maps + the grouped block plan from the mask."""
    n_tc = t_len // TCH
    n_dt = D // 128
    n_qc = t_len // QC

    def fuse(scale):
        s = np.asarray(scale, dtype=np.float32)
        c = np.asarray(cos, dtype=np.float32) * s[None, :]
        sw = np.empty_like(s)
        sw[0::2] = s[1::2]
        sw[1::2] = s[0::2]
        sn = np.asarray(sin, dtype=np.float32) * sw[None, :]
        sn[:, 0::2] *= -1.0
        return np.ascontiguousarray(c), np.ascontiguousarray(sn)

    cosq, sinq = fuse(qn_scale)
    cosk, sink = fuse(kn_scale)

    # mask block classification (mask: [1,1,T,T] bool, rows=q, cols=k)
    m2 = np.asarray(mask).reshape(t_len, t_len)
    plan = []
    mix_tiles = []
    for qci in range(n_qc):
        row = []
        qs = slice(qci * QC, (qci + 1) * QC)
        for kt in range(n_tc):
            blk = m2[qs, kt * 128:(kt + 1) * 128]
            if not blk.any():
                continue
            if blk.all():
                row.append((kt, -1))
            else:
                mix_tiles.append(np.ascontiguousarray(
                    blk.T.astype(np.float32)))
                row.append((kt, len(mix_tiles) - 1))
        groups = [row[i:i + KGRP] for i in range(0, len(row), KGRP)]
        plan.append(groups)
    maskp = (np.stack(mix_tiles) if mix_tiles
             else np.zeros((1, 128, QC), np.float32))

    ident = np.eye(128, dtype=np.float32)
    in_maps = []
    for c in range(NCORES):
        b, g = divmod(c, GROUPS)
        xt = np.asarray(x[b], dtype=np.float32).T  # [D, T]
        xhc = _trunc_f32r(
            xt.reshape(n_dt, 128, n_tc, TCH).transpose(2, 1, 0, 3)
            .reshape(n_tc, 128, n_dt * TCH))
        h0 = g * GROUPS
        in_maps.append({
            "xh": xhc,
            "wq": _trunc_f32r(
                np.asarray(wq, np.float32)[:, h0 * HD:(h0 + GROUPS) * HD]),
            "wkv": _trunc_f32r(np.concatenate(
                [np.asarray(wk, np.float32)[:, g * HD:(g + 1) * HD],
                 np.asarray(wv, np.float32)[:, g * HD:(g + 1) * HD]],
                axis=1)),
            "wo": _trunc_f32r(
                np.asarray(wo, np.float32)[h0 * HD:(h0 + GROUPS) * HD, :]),
            "cosq": cosq, "sinq": sinq, "cosk": cosk, "sink": sink,
            "ident": ident, "maskp": maskp,
        })
    return in_maps, plan, len(mix_tiles)


_NC_CACHE = {}


def _plan_key(plan, n_mixed, t_len, reps):
    return (t_len, n_mixed, reps,
            tuple(tuple(tuple(g) for g in r) for r in plan))


def kernel(x, wq, wk, wv, wo, qn_scale, kn_scale, cos, sin, mask):
    t_len = x.shape[1]
    in_maps, plan, n_mixed = _host_prep(
        x, wq, wk, wv, wo, qn_scale, kn_scale, cos, sin, mask, t_len)
    key = _plan_key(plan, n_mixed, t_len, 1)
    nc = _NC_CACHE.get(key)
    if nc is None:
        nc = _build_nc(t_len, plan, n_mixed)
        _NC_CACHE[key] = nc
    res = run_bass_kernel_spmd(nc, in_maps, list(range(NCORES)))
    out = np.zeros((B, t_len, D), dtype=np.float32)
    for c in range(NCORES):
        out[c // GROUPS] += res.results[c]["out_p"]
    return out


# revision 17
# speedup vs baseline: 192.6240x; 187.3162x over previous
"""Trainium2 Bass kernel for GQA attention block (B=2,T=2048,D=2048,H=16,KV=4,HD=128).

Sharding: 8 cores = 2 batches x 4 kv-groups. Core c handles batch b=c//4 and
kv-head g=c%4 (q-heads 4g..4g+3). wq/wk/wv column-sharded, wo row-sharded;
partial outputs are summed on the host (4 partials per batch).

Per-core dataflow (all matmuls in fp32r, 1 cycle/row; fp32r operands that
come from DRAM are pre-truncated on the host, which is idempotent under the
hardware rounding):
  phase 1: qT/kT/vT projections from host-pretransposed x, fused
           RMSNorm (ACT Square+accum, Sqrt, DVE recip) + RoPE
           (scalar_tensor_tensor with host-folded cos/sin tables incl.
           qn/kn scale and rotation sign), PE-transpose into [hd, t] layout.
  phase 2: per (head, q-chunk): scores^T = kT.T@qT (pairs of k-tiles into a
           2-bank PSUM tile) -> one ACT tanh + one ACT exp per pair ->
           optional mask multiply -> PE attn@v accumulate + ones-matmul row
           sums; normalize via DVE reciprocal + DRAM-bounce broadcast.
  phase 3: out^T heads @ wo row-block -> partial [T, D] output.

The causal/arbitrary mask is handled by host-side block classification:
all-false blocks are skipped, all-true blocks skip the mask multiply,
mixed blocks multiply by a packed 0/1 f32 tile.
"""

import math

import numpy as np

import concourse.bass as bass
import concourse.mybir as mybir
import concourse.tile as tile
from concourse import bacc
from concourse.bass_utils import run_bass_kernel_spmd

F32 = mybir.dt.float32
F32R = mybir.dt.float32r
AF = mybir.ActivationFunctionType
OP = mybir.AluOpType

B, T, D = 2, 2048, 2048
H, KV, HD = 16, 4, 128
GROUPS = H // KV
CAP = 50.0
EPS = 1e-6
NCORES = 8

TCH = 128  # t-chunk (phase-1 M, outproj M)
QC = 512   # q-chunk (phase-2 N); must be >=256 for fp32r full rate
KGRP = 2   # k-tiles per PSUM/ACT batch in phase 2


def _trunc_f32r(a):
    """Host-side fp32r pre-rounding (truncate low 11 mantissa bits)."""
    return (np.ascontiguousarray(a, dtype=np.float32)
            .view(np.uint32) & np.uint32(0xFFFFF800)).view(np.float32)


def _build_nc(t_len, plan, n_mixed, reps=1):
    """plan: per q-chunk, a list of groups; each group is a list of
    (kt, mixed_idx) pairs (mixed_idx=-1 for all-true blocks)."""
    n_tc = t_len // TCH
    n_dt = D // 128

    nc = bacc.Bacc("TRN2", target_bir_lowering=False, debug=False,
                   num_devices=NCORES)

    xh = nc.dram_tensor("xh", [n_tc, 128, n_dt * TCH], F32R,
                        kind="ExternalInput")
    wq_d = nc.dram_tensor("wq", [D, GROUPS * HD], F32R, kind="ExternalInput")
    wkv_d = nc.dram_tensor("wkv", [D, 2 * HD], F32R, kind="ExternalInput")
    wo_d = nc.dram_tensor("wo", [GROUPS * HD, D], F32R, kind="ExternalInput")
    cosq_d = nc.dram_tensor("cosq", [t_len, HD], F32, kind="ExternalInput")
    sinq_d = nc.dram_tensor("sinq", [t_len, HD], F32, kind="ExternalInput")
    cosk_d = nc.dram_tensor("cosk", [t_len, HD], F32, kind="ExternalInput")
    sink_d = nc.dram_tensor("sink", [t_len, HD], F32, kind="ExternalInput")
    ident_d = nc.dram_tensor("ident", [128, 128], F32, kind="ExternalInput")
    maskp_d = nc.dram_tensor("maskp", [max(n_mixed, 1), 128, QC], F32,
                             kind="ExternalInput")
    out_d = nc.dram_tensor("out_p", [t_len, D], F32, kind="ExternalOutput")
    rbounce_d = None

    inv_cap_scale = 1.0 / (CAP * math.sqrt(HD))

    with tile.TileContext(nc) as tc:
        for _rep in range(reps):
            _emit_body(nc, tc, t_len, plan, xh, wq_d, wkv_d, wo_d,
                       cosq_d, sinq_d, cosk_d, sink_d, ident_d, maskp_d,
                       out_d, rbounce_d, inv_cap_scale)
    nc.compile()
    return nc


def _emit_body(nc, tc, t_len, plan, xh, wq_d, wkv_d, wo_d, cosq_d, sinq_d,
               cosk_d, sink_d, ident_d, maskp_d, out_d, rbounce_d,
               inv_cap_scale):
    n_tc = t_len // TCH
    n_qc = t_len // QC
    n_dt = D // 128

    import contextlib
    with contextlib.ExitStack() as ctx:
        persist = ctx.enter_context(tc.tile_pool(name="persist", bufs=1))

        qT_all = persist.tile([128, GROUPS, t_len], F32R, tag="qT_all")
        kT_all = persist.tile([128, t_len], F32R, tag="kT_all")
        v_all = persist.tile([128, n_tc, HD], F32R, tag="v_all")
        ident = persist.tile([128, 128], F32, tag="ident")
        nc.sync.dma_start(out=ident, in_=ident_d.ap())
        ones_f = persist.tile([128, 1], F32, tag="ones_f")
        nc.vector.memset(ones_f, 1.0)
        ones_t = persist.tile([128, 1], F32R, tag="ones")
        nc.scalar.copy(out=ones_t, in_=ones_f)
        eps_t = persist.tile([128, 1], F32, tag="eps")
        nc.vector.memset(eps_t, EPS)

        # ---------------- phase 1: projections + norm + rope ----------
        with tc.tile_pool(name="w1", bufs=1) as w1, \
             tc.tile_pool(name="xstage", bufs=3) as xstage, \
             tc.tile_pool(name="cs", bufs=3) as cs, \
             tc.tile_pool(name="p1work", bufs=4) as p1work, \
             tc.tile_pool(name="psq", bufs=3, space="PSUM") as psq, \
             tc.tile_pool(name="pskv", bufs=2, space="PSUM") as pskv, \
             tc.tile_pool(name="pstr", bufs=2, space="PSUM") as pstr:

            wq_r = w1.tile([128, n_dt, GROUPS * HD], F32R, tag="wq")
            nc.sync.dma_start(
                out=wq_r,
                in_=wq_d.ap().rearrange("(dt p) n -> p dt n", p=128))
            wkv_r = w1.tile([128, n_dt, 2 * HD], F32R, tag="wkv")
            nc.sync.dma_start(
                out=wkv_r,
                in_=wkv_d.ap().rearrange("(dt p) n -> p dt n", p=128))

            def norm_rope_transpose(psrc, cos_t, sin_t, dst_slice):
                # psrc: PSUM [128, 128] (t x hd) raw projection for one head
                scratch = p1work.tile([128, HD], F32, tag="sqscr")
                ssq = p1work.tile([128, 1], F32, tag="ssq")
                nc.scalar.activation(scratch, psrc, AF.Square, accum_out=ssq)
                std = p1work.tile([128, 1], F32, tag="std")
                nc.scalar.activation(std, ssq, AF.Sqrt, scale=1.0 / HD,
                                     bias=eps_t)
                rinv = p1work.tile([128, 1], F32, tag="rinv")
                nc.vector.reciprocal(rinv, std)
                r1 = p1work.tile([128, HD], F32, tag="r1")
                nc.vector.scalar_tensor_tensor(r1, psrc, rinv, cos_t,
                                               op0=OP.mult, op1=OP.mult)
                qr = p1work.tile([128, HD], F32, tag="qr")
                p2 = psrc.rearrange("p (f two) -> p f two", two=2)
                s2 = sin_t.rearrange("p (f two) -> p f two", two=2)
                q2 = qr.rearrange("p (f two) -> p f two", two=2)
                nc.vector.scalar_tensor_tensor(q2[:, :, 0], p2[:, :, 1], rinv,
                                               s2[:, :, 0],
                                               op0=OP.mult, op1=OP.mult)
                nc.vector.scalar_tensor_tensor(q2[:, :, 1], p2[:, :, 0], rinv,
                                               s2[:, :, 1],
                                               op0=OP.mult, op1=OP.mult)
                nc.vector.tensor_add(qr, qr, r1)
                ptr = pstr.tile([128, 128], F32, tag="ptr")
                nc.tensor.transpose(ptr, qr, ident)
                nc.vector.tensor_copy(dst_slice, ptr)

            for tci in range(n_tc):
                xt = xstage.tile([128, n_dt * TCH], F32R, tag="xt")
                nc.sync.dma_start(out=xt, in_=xh.ap()[tci])
                xr = xt.rearrange("p (dt t) -> p dt t", dt=n_dt)

                pq = psq.tile([128, GROUPS * HD], F32, tag="pq")
                pkv = pskv.tile([128, 2 * HD], F32, tag="pkv")
                for dt in range(n_dt):
                    nc.tensor.matmul(pq, xr[:, dt, :], wq_r[:, dt, :],
                                     start=(dt == 0), stop=(dt == n_dt - 1))
                for dt in range(n_dt):
                    nc.tensor.matmul(pkv, xr[:, dt, :], wkv_r[:, dt, :],
                                     start=(dt == 0), stop=(dt == n_dt - 1))

                t0 = tci * TCH
                cq = cs.tile([128, HD], F32, tag="cq")
                nc.sync.dma_start(out=cq, in_=cosq_d.ap()[t0:t0 + TCH, :])
                sq_ = cs.tile([128, HD], F32, tag="sq")
                nc.sync.dma_start(out=sq_, in_=sinq_d.ap()[t0:t0 + TCH, :])
                ck = cs.tile([128, HD], F32, tag="ck")
                nc.sync.dma_start(out=ck, in_=cosk_d.ap()[t0:t0 + TCH, :])
                sk = cs.tile([128, HD], F32, tag="sk")
                nc.sync.dma_start(out=sk, in_=sink_d.ap()[t0:t0 + TCH, :])

                for h in range(GROUPS):
                    norm_rope_transpose(pq[:, h * HD:(h + 1) * HD], cq, sq_,
                                        qT_all[:, h, t0:t0 + TCH])
                norm_rope_transpose(pkv[:, 0:HD], ck, sk,
                                    kT_all[:, t0:t0 + TCH])
                nc.scalar.copy(out=v_all[:, tci, :], in_=pkv[:, HD:2 * HD])

        # ---------------- phase 2+3: attention + output projection ----
        with tc.tile_pool(name="w2", bufs=1) as w2, \
             tc.tile_pool(name="p2work", bufs=4) as p2work, \
             tc.tile_pool(name="pTpool", bufs=6) as pTpool, \
             tc.tile_pool(name="mstage", bufs=2) as mstage, \
             tc.tile_pool(name="ohpool", bufs=2) as ohpool, \
             tc.tile_pool(name="norm2", bufs=2) as norm2, \
             tc.tile_pool(name="obuf", bufs=4) as obuf, \
             tc.tile_pool(name="pss", bufs=2, space="PSUM") as pss, \
             tc.tile_pool(name="psav", bufs=2, space="PSUM") as psav, \
             tc.tile_pool(name="psrs", bufs=1, space="PSUM") as psrs, \
             tc.tile_pool(name="psop", bufs=1, space="PSUM") as psop:

            wo_r = w2.tile([128, GROUPS, D], F32R, tag="wo")
            nc.sync.dma_start(
                out=wo_r,
                in_=wo_d.ap().rearrange("(h p) n -> p h n", p=128))

            for qci in range(n_qc):
                q0 = qci * QC
                oh_all = ohpool.tile([128, GROUPS, QC], F32R, tag="oh")
                for h in range(GROUPS):
                    groups_ = plan[qci]
                    n_kt = sum(len(g) for g in groups_)
                    pav = psav.tile([128, QC], F32, tag="pav")
                    prs = psrs.tile([1, QC], F32, tag="prs")
                    gidx = 0
                    for grp in groups_:
                        gw = len(grp)
                        ps = pss.tile([128, KGRP * QC], F32, tag="ps")
                        for j, (kt, mix) in enumerate(grp):
                            nc.tensor.matmul(
                                ps[:, j * QC:(j + 1) * QC],
                                kT_all[:, kt * 128:(kt + 1) * 128],
                                qT_all[:, h, q0:q0 + QC],
                                start=True, stop=True)
                        pview = ps[:, 0:gw * QC]
                        th = p2work.tile([128, KGRP * QC], F32, tag="th")
                        thv = th[:, 0:gw * QC]
                        nc.scalar.activation(thv, pview, AF.Tanh,
                                             scale=inv_cap_scale)
                        any_mix = any(mix >= 0 for (_, mix) in grp)
                        pt = pTpool.tile([128, KGRP * QC], F32R, tag="pt")
                        if not any_mix:
                            nc.scalar.activation(pt[:, 0:gw * QC], thv,
                                                 AF.Exp, scale=CAP)
                        else:
                            ptf = p2work.tile([128, KGRP * QC], F32,
                                              tag="ptf")
                            nc.scalar.activation(ptf[:, 0:gw * QC], thv,
                                                 AF.Exp, scale=CAP)
                            for j, (kt, mix) in enumerate(grp):
                                sl = slice(j * QC, (j + 1) * QC)
                                if mix >= 0:
                                    mt = mstage.tile([128, QC], F32,
                                                     tag="mt")
                                    nc.sync.dma_start(
                                        out=mt, in_=maskp_d.ap()[mix])
                                    nc.vector.tensor_tensor(
                                        pt[:, sl], ptf[:, sl], mt,
                                        op=OP.mult)
                                else:
                                    nc.vector.tensor_copy(
                                        pt[:, sl], ptf[:, sl])
                        for j, (kt, mix) in enumerate(grp):
                            sl = slice(j * QC, (j + 1) * QC)
                            first = gidx == 0
                            last = gidx == n_kt - 1
                            nc.tensor.matmul(pav, v_all[:, kt, :],
                                             pt[:, sl],
                                             start=first, stop=last)
                            nc.tensor.matmul(prs, ones_t, pt[:, sl],
                                             start=first, stop=last)
                            gidx += 1
                    # normalize: oh = pav * bcast(1/rowsum)
                    rin = norm2.tile([1, QC], F32, tag="rin")
                    nc.vector.reciprocal(rin, prs)
                    rbs = norm2.tile([128, QC], F32, tag="rbs")
                    nc.gpsimd.partition_broadcast(rbs, rin)
                    nc.vector.tensor_tensor(oh_all[:, h, :], pav, rbs,
                                            op=OP.mult)

                # outproj for this q-chunk
                for tsub in range(QC // TCH):
                    for dc in range(D // 512):
                        pop = psop.tile([128, 512], F32, tag="pop")
                        for h in range(GROUPS):
                            nc.tensor.matmul(
                                pop,
                                oh_all[:, h, tsub * TCH:(tsub + 1) * TCH],
                                wo_r[:, h, dc * 512:(dc + 1) * 512],
                                start=(h == 0), stop=(h == GROUPS - 1))
                        ot = obuf.tile([128, 512], F32, tag="ot")
                        nc.vector.tensor_copy(ot, pop)
                        r0 = q0 + tsub * TCH
                        nc.sync.dma_start(
                            out=out_d.ap()[r0:r0 + TCH,
                                           dc * 512:(dc + 1) * 512],
                            in_=ot)


def _host_prep(x, wq, wk, wv, wo, qn_scale, kn_scale, cos, sin, mask, t_len):
    """Build per-core input maps + the grouped block plan from the mask."""
    n_tc = t_len // TCH
    n_dt = D // 128
    n_qc = t_len // QC

    def fuse(scale):
        s = np.asarray(scale, dtype=np.float32)
        c = np.asarray(cos, dtype=np.float32) * s[None, :]
        sw = np.empty_like(s)
        sw[0::2] = s[1::2]
        sw[1::2] = s[0::2]
        sn = np.asarray(sin, dtype=np.float32) * sw[None, :]
        sn[:, 0::2] *= -1.0
        return np.ascontiguousarray(c), np.ascontiguousarray(sn)

    cosq, sinq = fuse(qn_scale)
    cosk, sink = fuse(kn_scale)

    # mask block classification (mask: [1,1,T,T] bool, rows=q, cols=k)
    m2 = np.asarray(mask).reshape(t_len, t_len)
    plan = []
    mix_tiles = []
    for qci in range(n_qc):
        row = []
        qs = slice(qci * QC, (qci + 1) * QC)
        for kt in range(n_tc):
            blk = m2[qs, kt * 128:(kt + 1) * 128]
            if not blk.any():
                continue
            if blk.all():
                row.append((kt, -1))
            else:
                mix_tiles.append(np.ascontiguousarray(
                    blk.T.astype(np.float32)))
                row.append((kt, len(mix_tiles) - 1))
        groups = [row[i:i + KGRP] for i in range(0, len(row), KGRP)]
        plan.append(groups)
    maskp = (np.stack(mix_tiles) if mix_tiles
             else np.zeros((1, 128, QC), np.float32))

    ident = np.eye(128, dtype=np.float32)
    in_maps = []
    for c in range(NCORES):
        b, g = divmod(c, GROUPS)
        xt = np.asarray(x[b], dtype=np.float32).T  # [D, T]
        xhc = _trunc_f32r(
            xt.reshape(n_dt, 128, n_tc, TCH).transpose(2, 1, 0, 3)
            .reshape(n_tc, 128, n_dt * TCH))
        h0 = g * GROUPS
        in_maps.append({
            "xh": xhc,
            "wq": _trunc_f32r(
                np.asarray(wq, np.float32)[:, h0 * HD:(h0 + GROUPS) * HD]),
            "wkv": _trunc_f32r(np.concatenate(
                [np.asarray(wk, np.float32)[:, g * HD:(g + 1) * HD],
                 np.asarray(wv, np.float32)[:, g * HD:(g + 1) * HD]],
                axis=1)),
            "wo": _trunc_f32r(
                np.asarray(wo, np.float32)[h0 * HD:(h0 + GROUPS) * HD, :]),
            "cosq": cosq, "sinq": sinq, "cosk": cosk, "sink": sink,
            "ident": ident, "maskp": maskp,
        })
    return in_maps, plan, len(mix_tiles)


_NC_CACHE = {}


def _plan_key(plan, n_mixed, t_len, reps):
    return (t_len, n_mixed, reps,
            tuple(tuple(tuple(g) for g in r) for r in plan))


def kernel(x, wq, wk, wv, wo, qn_scale, kn_scale, cos, sin, mask):
    t_len = x.shape[1]
    in_maps, plan, n_mixed = _host_prep(
        x, wq, wk, wv, wo, qn_scale, kn_scale, cos, sin, mask, t_len)
    key = _plan_key(plan, n_mixed, t_len, 1)
    nc = _NC_CACHE.get(key)
    if nc is None:
        nc = _build_nc(t_len, plan, n_mixed)
        _NC_CACHE[key] = nc
    res = run_bass_kernel_spmd(nc, in_maps, list(range(NCORES)))
    out = np.zeros((B, t_len, D), dtype=np.float32)
    for c in range(NCORES):
        out[c // GROUPS] += res.results[c]["out_p"]
    return out


# revision 18
# speedup vs baseline: 260.8119x; 1.3540x over previous
"""Trainium2 Bass kernel for GQA attention block (B=2,T=2048,D=2048,H=16,KV=4,HD=128).

Sharding: 8 cores = 2 batches x 4 kv-groups. Core c handles batch b=c//4 and
kv-head g=c%4 (q-heads 4g..4g+3). wq/wk/wv column-sharded, wo row-sharded;
partial outputs are summed on the host (4 partials per batch).

Per-core dataflow (all matmuls in fp32r, 1 cycle/row; fp32r operands that
come from DRAM are pre-truncated on the host, which is idempotent under the
hardware rounding):
  phase 1: qT/kT/vT projections from host-pretransposed x, fused
           RMSNorm (ACT Square+accum, Sqrt, DVE recip) + RoPE
           (scalar_tensor_tensor with host-folded cos/sin tables incl.
           qn/kn scale and rotation sign), PE-transpose into [hd, t] layout.
  phase 2: per (head, q-chunk): scores^T = kT.T@qT (pairs of k-tiles into a
           2-bank PSUM tile) -> one ACT tanh + one ACT exp per pair ->
           optional mask multiply -> PE attn@v accumulate + ones-matmul row
           sums; normalize via DVE reciprocal + DRAM-bounce broadcast.
  phase 3: out^T heads @ wo row-block -> partial [T, D] output.

The causal/arbitrary mask is handled by host-side block classification:
all-false blocks are skipped, all-true blocks skip the mask multiply,
mixed blocks multiply by a packed 0/1 f32 tile.
"""

import math
import os

# must be set before the axon/jax client initializes: recovers wedged cores
os.environ.setdefault("NEURON_RT_RESET_CORES", "1")

import numpy as np

import concourse.bass as bass
import concourse.mybir as mybir
import concourse.tile as tile
from concourse import bacc
from concourse.bass_utils import run_bass_kernel_spmd

F32 = mybir.dt.float32
F32R = mybir.dt.float32r
AF = mybir.ActivationFunctionType
OP = mybir.AluOpType

B, T, D = 2, 2048, 2048
H, KV, HD = 16, 4, 128
GROUPS = H // KV
CAP = 50.0
EPS = 1e-6
NCORES = 8

TCH = 128  # t-chunk (phase-1 M, outproj M)
QC = 512   # q-chunk (phase-2 N); must be >=256 for fp32r full rate
KGRP = 2   # k-tiles per PSUM/ACT batch in phase 2


def _trunc_f32r(a):
    """Host-side fp32r pre-rounding (truncate low 11 mantissa bits)."""
    return (np.ascontiguousarray(a, dtype=np.float32)
            .view(np.uint32) & np.uint32(0xFFFFF800)).view(np.float32)


def _build_nc(t_len, plan, n_mixed, reps=1):
    """plan: per q-chunk, a list of groups; each group is a list of
    (kt, mixed_idx) pairs (mixed_idx=-1 for all-true blocks)."""
    n_tc = t_len // TCH
    n_dt = D // 128

    nc = bacc.Bacc("TRN2", target_bir_lowering=False, debug=False,
                   num_devices=NCORES)

    xh = nc.dram_tensor("xh", [n_tc, 128, n_dt * TCH], F32R,
                        kind="ExternalInput")
    wq_d = nc.dram_tensor("wq", [D, GROUPS * HD], F32R, kind="ExternalInput")
    wkv_d = nc.dram_tensor("wkv", [D, 2 * HD], F32R, kind="ExternalInput")
    wo_d = nc.dram_tensor("wo", [GROUPS * HD, D], F32R, kind="ExternalInput")
    cosq_d = nc.dram_tensor("cosq", [t_len, HD], F32, kind="ExternalInput")
    sinq_d = nc.dram_tensor("sinq", [t_len, HD], F32, kind="ExternalInput")
    cosk_d = nc.dram_tensor("cosk", [t_len, HD], F32, kind="ExternalInput")
    sink_d = nc.dram_tensor("sink", [t_len, HD], F32, kind="ExternalInput")
    ident_d = nc.dram_tensor("ident", [128, 128], F32, kind="ExternalInput")
    maskp_d = nc.dram_tensor("maskp", [max(n_mixed, 1), 128, QC], F32,
                             kind="ExternalInput")
    out_d = nc.dram_tensor("out_p", [t_len, D], F32, kind="ExternalOutput")
    rbounce_d = None

    inv_cap_scale = 1.0 / (CAP * math.sqrt(HD))

    with tile.TileContext(nc) as tc:
        for _rep in range(reps):
            _emit_body(nc, tc, t_len, plan, xh, wq_d, wkv_d, wo_d,
                       cosq_d, sinq_d, cosk_d, sink_d, ident_d, maskp_d,
                       out_d, rbounce_d, inv_cap_scale)
    nc.compile()
    return nc


def _emit_body(nc, tc, t_len, plan, xh, wq_d, wkv_d, wo_d, cosq_d, sinq_d,
               cosk_d, sink_d, ident_d, maskp_d, out_d, rbounce_d,
               inv_cap_scale):
    n_tc = t_len // TCH
    n_qc = t_len // QC
    n_dt = D // 128

    import contextlib
    with contextlib.ExitStack() as ctx:
        persist = ctx.enter_context(tc.tile_pool(name="persist", bufs=1))

        qT_all = persist.tile([128, GROUPS, t_len], F32R, tag="qT_all")
        kT_all = persist.tile([128, t_len], F32R, tag="kT_all")
        v_all = persist.tile([128, n_tc, HD], F32R, tag="v_all")
        ident = persist.tile([128, 128], F32, tag="ident")
        nc.sync.dma_start(out=ident, in_=ident_d.ap())
        ones_f = persist.tile([128, 1], F32, tag="ones_f")
        nc.vector.memset(ones_f, 1.0)
        ones_t = persist.tile([128, 1], F32R, tag="ones")
        nc.scalar.copy(out=ones_t, in_=ones_f)
        eps_t = persist.tile([128, 1], F32, tag="eps")
        nc.vector.memset(eps_t, EPS)

        # ---------------- phase 1: projections + norm + rope ----------
        with tc.tile_pool(name="w1", bufs=1) as w1, \
             tc.tile_pool(name="xstage", bufs=3) as xstage, \
             tc.tile_pool(name="cs", bufs=3) as cs, \
             tc.tile_pool(name="p1work", bufs=4) as p1work, \
             tc.tile_pool(name="psq", bufs=3, space="PSUM") as psq, \
             tc.tile_pool(name="pskv", bufs=2, space="PSUM") as pskv, \
             tc.tile_pool(name="pstr", bufs=2, space="PSUM") as pstr:

            wq_r = w1.tile([128, n_dt, GROUPS * HD], F32R, tag="wq")
            nc.sync.dma_start(
                out=wq_r,
                in_=wq_d.ap().rearrange("(dt p) n -> p dt n", p=128))
            wkv_r = w1.tile([128, n_dt, 2 * HD], F32R, tag="wkv")
            nc.sync.dma_start(
                out=wkv_r,
                in_=wkv_d.ap().rearrange("(dt p) n -> p dt n", p=128))

            def norm_rope_transpose(psrc, cos_t, sin_t, dst_slice):
                # psrc: PSUM [128, 128] (t x hd) raw projection for one head
                scratch = p1work.tile([128, HD], F32, tag="sqscr")
                ssq = p1work.tile([128, 1], F32, tag="ssq")
                nc.scalar.activation(scratch, psrc, AF.Square, accum_out=ssq)
                std = p1work.tile([128, 1], F32, tag="std")
                nc.scalar.activation(std, ssq, AF.Sqrt, scale=1.0 / HD,
                                     bias=eps_t)
                rinv = p1work.tile([128, 1], F32, tag="rinv")
                nc.vector.reciprocal(rinv, std)
                r1 = p1work.tile([128, HD], F32, tag="r1")
                nc.vector.scalar_tensor_tensor(r1, psrc, rinv, cos_t,
                                               op0=OP.mult, op1=OP.mult)
                qr = p1work.tile([128, HD], F32, tag="qr")
                p2 = psrc.rearrange("p (f two) -> p f two", two=2)
                s2 = sin_t.rearrange("p (f two) -> p f two", two=2)
                q2 = qr.rearrange("p (f two) -> p f two", two=2)
                nc.vector.scalar_tensor_tensor(q2[:, :, 0], p2[:, :, 1], rinv,
                                               s2[:, :, 0],
                                               op0=OP.mult, op1=OP.mult)
                nc.vector.scalar_tensor_tensor(q2[:, :, 1], p2[:, :, 0], rinv,
                                               s2[:, :, 1],
                                               op0=OP.mult, op1=OP.mult)
                nc.vector.tensor_add(qr, qr, r1)
                ptr = pstr.tile([128, 128], F32, tag="ptr")
                nc.tensor.transpose(ptr, qr, ident)
                nc.vector.tensor_copy(dst_slice, ptr)

            for tci in range(n_tc):
                xt = xstage.tile([128, n_dt * TCH], F32R, tag="xt")
                nc.sync.dma_start(out=xt, in_=xh.ap()[tci])
                xr = xt.rearrange("p (dt t) -> p dt t", dt=n_dt)

                pq = psq.tile([128, GROUPS * HD], F32, tag="pq")
                pkv = pskv.tile([128, 2 * HD], F32, tag="pkv")
                for dt in range(n_dt):
                    nc.tensor.matmul(pq, xr[:, dt, :], wq_r[:, dt, :],
                                     start=(dt == 0), stop=(dt == n_dt - 1))
                for dt in range(n_dt):
                    nc.tensor.matmul(pkv, xr[:, dt, :], wkv_r[:, dt, :],
                                     start=(dt == 0), stop=(dt == n_dt - 1))

                t0 = tci * TCH
                cq = cs.tile([128, HD], F32, tag="cq")
                nc.sync.dma_start(out=cq, in_=cosq_d.ap()[t0:t0 + TCH, :])
                sq_ = cs.tile([128, HD], F32, tag="sq")
                nc.sync.dma_start(out=sq_, in_=sinq_d.ap()[t0:t0 + TCH, :])
                ck = cs.tile([128, HD], F32, tag="ck")
                nc.sync.dma_start(out=ck, in_=cosk_d.ap()[t0:t0 + TCH, :])
                sk = cs.tile([128, HD], F32, tag="sk")
                nc.sync.dma_start(out=sk, in_=sink_d.ap()[t0:t0 + TCH, :])

                for h in range(GROUPS):
                    norm_rope_transpose(pq[:, h * HD:(h + 1) * HD], cq, sq_,
                                        qT_all[:, h, t0:t0 + TCH])
                norm_rope_transpose(pkv[:, 0:HD], ck, sk,
                                    kT_all[:, t0:t0 + TCH])
                nc.scalar.copy(out=v_all[:, tci, :], in_=pkv[:, HD:2 * HD])

        # ---------------- phase 2+3: attention + output projection ----
        with tc.tile_pool(name="w2", bufs=1) as w2, \
             tc.tile_pool(name="p2work", bufs=4) as p2work, \
             tc.tile_pool(name="pTpool", bufs=6) as pTpool, \
             tc.tile_pool(name="mstage", bufs=2) as mstage, \
             tc.tile_pool(name="ohpool", bufs=2) as ohpool, \
             tc.tile_pool(name="norm2", bufs=2) as norm2, \
             tc.tile_pool(name="obuf", bufs=4) as obuf, \
             tc.tile_pool(name="pss", bufs=2, space="PSUM") as pss, \
             tc.tile_pool(name="psav", bufs=2, space="PSUM") as psav, \
             tc.tile_pool(name="psrs", bufs=1, space="PSUM") as psrs, \
             tc.tile_pool(name="psop", bufs=1, space="PSUM") as psop:

            wo_r = w2.tile([128, GROUPS, D], F32R, tag="wo")
            nc.sync.dma_start(
                out=wo_r,
                in_=wo_d.ap().rearrange("(h p) n -> p h n", p=128))

            for qci in range(n_qc):
                q0 = qci * QC
                oh_all = ohpool.tile([128, GROUPS, QC], F32R, tag="oh")
                for h in range(GROUPS):
                    groups_ = plan[qci]
                    n_kt = sum(len(g) for g in groups_)
                    pav = psav.tile([128, QC], F32, tag="pav")
                    prs = psrs.tile([1, QC], F32, tag="prs")
                    gidx = 0
                    for grp in groups_:
                        gw = len(grp)
                        ps = pss.tile([128, KGRP * QC], F32, tag="ps")
                        for j, (kt, mix) in enumerate(grp):
                            nc.tensor.matmul(
                                ps[:, j * QC:(j + 1) * QC],
                                kT_all[:, kt * 128:(kt + 1) * 128],
                                qT_all[:, h, q0:q0 + QC],
                                start=True, stop=True)
                        pview = ps[:, 0:gw * QC]
                        th = p2work.tile([128, KGRP * QC], F32, tag="th")
                        thv = th[:, 0:gw * QC]
                        nc.scalar.activation(thv, pview, AF.Tanh,
                                             scale=inv_cap_scale)
                        any_mix = any(mix >= 0 for (_, mix) in grp)
                        pt = pTpool.tile([128, KGRP * QC], F32R, tag="pt")
                        if not any_mix:
                            nc.scalar.activation(pt[:, 0:gw * QC], thv,
                                                 AF.Exp, scale=CAP)
                        else:
                            ptf = p2work.tile([128, KGRP * QC], F32,
                                              tag="ptf")
                            nc.scalar.activation(ptf[:, 0:gw * QC], thv,
                                                 AF.Exp, scale=CAP)
                            for j, (kt, mix) in enumerate(grp):
                                sl = slice(j * QC, (j + 1) * QC)
                                if mix >= 0:
                                    mt = mstage.tile([128, QC], F32,
                                                     tag="mt")
                                    nc.sync.dma_start(
                                        out=mt, in_=maskp_d.ap()[mix])
                                    nc.vector.tensor_tensor(
                                        pt[:, sl], ptf[:, sl], mt,
                                        op=OP.mult)
                                else:
                                    nc.vector.tensor_copy(
                                        pt[:, sl], ptf[:, sl])
                        for j, (kt, mix) in enumerate(grp):
                            sl = slice(j * QC, (j + 1) * QC)
                            first = gidx == 0
                            last = gidx == n_kt - 1
                            nc.tensor.matmul(pav, v_all[:, kt, :],
                                             pt[:, sl],
                                             start=first, stop=last)
                            nc.tensor.matmul(prs, ones_t, pt[:, sl],
                                             start=first, stop=last)
                            gidx += 1
                    # normalize: oh = pav * bcast(1/rowsum)
                    rin = norm2.tile([1, QC], F32, tag="rin")
                    nc.vector.reciprocal(rin, prs)
                    rbs = norm2.tile([128, QC], F32, tag="rbs")
                    nc.gpsimd.partition_broadcast(rbs, rin)
                    nc.vector.tensor_tensor(oh_all[:, h, :], pav, rbs,
                                            op=OP.mult)

                # outproj for this q-chunk
                for tsub in range(QC // TCH):
                    for dc in range(D // 512):
                        pop = psop.tile([128, 512], F32, tag="pop")
                        for h in range(GROUPS):
                            nc.tensor.matmul(
                                pop,
                                oh_all[:, h, tsub * TCH:(tsub + 1) * TCH],
                                wo_r[:, h, dc * 512:(dc + 1) * 512],
                                start=(h == 0), stop=(h == GROUPS - 1))
                        ot = obuf.tile([128, 512], F32, tag="ot")
                        nc.vector.tensor_copy(ot, pop)
                        r0 = q0 + tsub * TCH
                        nc.sync.dma_start(
                            out=out_d.ap()[r0:r0 + TCH,
                                           dc * 512:(dc + 1) * 512],
                            in_=ot)


def _host_prep(x, wq, wk, wv, wo, qn_scale, kn_scale, cos, sin, mask, t_len):
    """Build per-core input maps + the grouped block plan from the mask."""
    n_tc = t_len // TCH
    n_dt = D // 128
    n_qc = t_len // QC

    def fuse(scale):
        s = np.asarray(scale, dtype=np.float32)
        c = np.asarray(cos, dtype=np.float32) * s[None, :]
        sw = np.empty_like(s)
        sw[0::2] = s[1::2]
        sw[1::2] = s[0::2]
        sn = np.asarray(sin, dtype=np.float32) * sw[None, :]
        sn[:, 0::2] *= -1.0
        return np.ascontiguousarray(c), np.ascontiguousarray(sn)

    cosq, sinq = fuse(qn_scale)
    cosk, sink = fuse(kn_scale)

    # mask block classification (mask: [1,1,T,T] bool, rows=q, cols=k)
    m2 = np.asarray(mask).reshape(t_len, t_len)
    plan = []
    mix_tiles = []
    for qci in range(n_qc):
        row = []
        qs = slice(qci * QC, (qci + 1) * QC)
        for kt in range(n_tc):
            blk = m2[qs, kt * 128:(kt + 1) * 128]
            if not blk.any():
                continue
            if blk.all():
                row.append((kt, -1))
            else:
                mix_tiles.append(np.ascontiguousarray(
                    blk.T.astype(np.float32)))
                row.append((kt, len(mix_tiles) - 1))
        groups = [row[i:i + KGRP] for i in range(0, len(row), KGRP)]
        plan.append(groups)
    maskp = (np.stack(mix_tiles) if mix_tiles
             else np.zeros((1, 128, QC), np.float32))

    ident = np.eye(128, dtype=np.float32)
    in_maps = []
    for c in range(NCORES):
        b, g = divmod(c, GROUPS)
        xt = np.asarray(x[b], dtype=np.float32).T  # [D, T]
        xhc = _trunc_f32r(
            xt.reshape(n_dt, 128, n_tc, TCH).transpose(2, 1, 0, 3)
            .reshape(n_tc, 128, n_dt * TCH))
        h0 = g * GROUPS
        in_maps.append({
            "xh": xhc,
            "wq": _trunc_f32r(
                np.asarray(wq, np.float32)[:, h0 * HD:(h0 + GROUPS) * HD]),
            "wkv": _trunc_f32r(np.concatenate(
                [np.asarray(wk, np.float32)[:, g * HD:(g + 1) * HD],
                 np.asarray(wv, np.float32)[:, g * HD:(g + 1) * HD]],
                axis=1)),
            "wo": _trunc_f32r(
                np.asarray(wo, np.float32)[h0 * HD:(h0 + GROUPS) * HD, :]),
            "cosq": cosq, "sinq": sinq, "cosk": cosk, "sink": sink,
            "ident": ident, "maskp": maskp,
        })
    return in_maps, plan, len(mix_tiles)


_NC_CACHE = {}


def _plan_key(plan, n_mixed, t_len, reps):
    return (t_len, n_mixed, reps,
            tuple(tuple(tuple(g) for g in r) for r in plan))


def kernel(x, wq, wk, wv, wo, qn_scale, kn_scale, cos, sin, mask):
    t_len = x.shape[1]
    in_maps, plan, n_mixed = _host_prep(
        x, wq, wk, wv, wo, qn_scale, kn_scale, cos, sin, mask, t_len)
    key = _plan_key(plan, n_mixed, t_len, 1)
    nc = _NC_CACHE.get(key)
    if nc is None:
        nc = _build_nc(t_len, plan, n_mixed)
        _NC_CACHE[key] = nc
    try:
        res = run_bass_kernel_spmd(nc, in_maps, list(range(NCORES)))
    except Exception:
        # one retry: transient device faults (e.g. a wedged core from a
        # previous process) usually clear on re-execution with
        # NEURON_RT_RESET_CORES=1
        res = run_bass_kernel_spmd(nc, in_maps, list(range(NCORES)))
    out = np.zeros((B, t_len, D), dtype=np.float32)
    for c in range(NCORES):
        out[c // GROUPS] += res.results[c]["out_p"]
    return out
